# revision 6
# baseline (speedup 1.0000x reference)
"""Trainium2 Bass kernel for nn_ArthDenseCalcToDenseBlock.

The reference is a 256-step sequential scan per batch row, but the state
machine freezes at the first valid operator token (the `meet` gate), so the
whole scan collapses to closed-form masked reductions along the sequence
axis, computed per row with DVE prefix-scan instructions:

  meet[j] = running-max of (valid-op mask)        -> first-op one-hot, met
  cpre[j] = running-sum of (valid-number mask)    -> last / 2nd-last number
                                                     one-hots via == total
  h0,h1,vmax,v_c gathered with masked-sum accumulations; one scatter back.

Data parallel over batch: 4096 rows -> 8 cores x 512 rows -> 4 tiles of 128
partitions. trans_op is host-padded to 8 channels (with -inf) so the
channel-max reduce hits the DVE 2x perf mode.
"""

from contextlib import ExitStack

import numpy as np

import concourse.bacc as bacc
import concourse.bass as bass
import concourse.mybir as mybir
import concourse.tile as tile
from concourse.bass_utils import run_bass_kernel_spmd

F32 = mybir.dt.float32
OP = mybir.AluOpType
ACTF = mybir.ActivationFunctionType
AX = mybir.AxisListType

B, S, NOPS = 4096, 256, 7
C8 = 8                    # padded channel count
NCORES = 8
BS = B // NCORES          # rows per core
P = 128                   # partitions
NT = BS // P              # row-tiles per core


def _build_nc():
    nc = bacc.Bacc("TRN2", target_bir_lowering=False, debug=False)

    tv_d = nc.dram_tensor("tv", [BS, S], F32, kind="ExternalInput")
    td_d = nc.dram_tensor("td", [BS, S], F32, kind="ExternalInput")
    op_d = nc.dram_tensor("op", [BS, S * C8], F32, kind="ExternalInput")
    fin_d = nc.dram_tensor("fin", [BS], F32, kind="ExternalInput")
    val_d = nc.dram_tensor("val", [BS], F32, kind="ExternalInput")
    act_d = nc.dram_tensor("act", [S], F32, kind="ExternalInput")

    tvo_d = nc.dram_tensor("tv_out", [BS, S], F32, kind="ExternalOutput")
    tdo_d = nc.dram_tensor("td_out", [BS, S], F32, kind="ExternalOutput")
    iff_d = nc.dram_tensor("iff_out", [BS], F32, kind="ExternalOutput")
    ivo_d = nc.dram_tensor("iv_out", [BS], F32, kind="ExternalOutput")

    with tile.TileContext(nc) as tc, ExitStack() as ctx:
        cpool = ctx.enter_context(tc.tile_pool(name="consts", bufs=1))
        io_pool = ctx.enter_context(tc.tile_pool(name="io", bufs=NT + 1))
        op_pool = ctx.enter_context(tc.tile_pool(name="op", bufs=2))
        work = ctx.enter_context(tc.tile_pool(name="work", bufs=3))
        keep = ctx.enter_context(tc.tile_pool(name="keep", bufs=NT + 1))
        gsc = ctx.enter_context(tc.tile_pool(name="gsc", bufs=4))
        sm = ctx.enter_context(tc.tile_pool(name="small", bufs=1))

        # ---- constants
        crow = cpool.tile([P, S], F32)
        nc.sync.dma_start(crow[0:1, :], act_d.ap().rearrange("(o s) -> o s", o=1))
        act_bc = cpool.tile([P, S], F32)
        nc.gpsimd.partition_broadcast(act_bc[:], crow[0:1, :])
        zero_bc = cpool.tile([P, S], F32)
        nc.vector.memset(zero_bc[:], 0.0)

        # ---- per-row gates: [P, NT], column t = rows [t*128, (t+1)*128)
        fin_t = sm.tile([P, NT], F32)
        val_t = sm.tile([P, NT], F32)
        nc.sync.dma_start(fin_t[:], fin_d.ap().rearrange("(t p) -> p t", p=P))
        nc.sync.dma_start(val_t[:], val_d.ap().rearrange("(t p) -> p t", p=P))
        omf = sm.tile([P, NT], F32)   # 1 - f
        nc.vector.tensor_scalar(omf[:], fin_t[:], -1.0, 1.0, op0=OP.mult, op1=OP.add)
        gate = sm.tile([P, NT], F32)  # (1 - f) * g
        nc.vector.tensor_mul(gate[:], omf[:], val_t[:])

        # ---- batched per-row scalars, column t = row-tile t
        met_a = sm.tile([P, NT], F32)
        cnt1_a = sm.tile([P, NT], F32)
        cnt2_a = sm.tile([P, NT], F32)
        fire_a = sm.tile([P, NT], F32)
        h0_a = sm.tile([P, NT], F32)
        h1_a = sm.tile([P, NT], F32)
        vmax_a = sm.tile([P, NT], F32)
        vc_a = [sm.tile([P, NT], F32, name=f"vc{c}") for c in range(2, 7)]

        tv_tiles, td_tiles, ohp0_tiles, ohp1_tiles, ohi_tiles = [], [], [], [], []

        # =========================== phase A ===========================
        for t in range(NT):
            rows = slice(t * P, (t + 1) * P)
            tvt = io_pool.tile([P, S], F32, tag="tvt")
            tdt = io_pool.tile([P, S], F32, tag="tdt")
            opt = op_pool.tile([P, S * C8], F32)
            nc.sync.dma_start(tvt[:], tv_d[rows, :])
            nc.sync.dma_start(tdt[:], td_d[rows, :])
            nc.sync.dma_start(opt[:], op_d[rows, :])
            opv = opt[:].rearrange("p (s c) -> p s c", c=C8)

            gate_t = gate[:, t : t + 1]

            # channel max over all 8 (ch7 = -inf pad); is_op = (m7 > ch0)
            m7 = work.tile([P, S], F32, tag="m7")
            nc.vector.tensor_reduce(m7[:], opv[:, :, 0:C8], axis=AX.X, op=OP.max)
            is_op = work.tile([P, S], F32, tag="isop")
            nc.any.tensor_tensor(is_op[:], m7[:], opv[:, :, 0], op=OP.is_gt)

            tvg = work.tile([P, S], F32, tag="tvg")
            nc.vector.scalar_tensor_tensor(
                tvg[:], tvt[:], gate_t, act_bc[:], op0=OP.mult, op1=OP.mult)
            vop = work.tile([P, S], F32, tag="vop")
            nc.gpsimd.tensor_mul(vop[:], tvg[:], is_op[:])

            # meet[j] = running max of vop  (inclusive)
            meet = work.tile([P, S], F32, tag="meet")
            nc.vector.tensor_tensor_scan(
                meet[:], vop[:], vop[:], 0.0, op0=OP.max, op1=OP.max)
            met = meet[:, S - 1 : S]
            nc.vector.tensor_copy(met_a[:, t : t + 1], met)

            # first-op one-hot: vop & ~meet_exclusive
            ohi = keep.tile([P, S], F32, tag="ohi")
            nc.vector.scalar_tensor_tensor(
                ohi[:, 1:S], meet[:, 0 : S - 1], 1.0, vop[:, 1:S],
                op0=OP.not_equal, op1=OP.mult)
            nc.vector.tensor_copy(ohi[:, 0:1], vop[:, 0:1])

            # valid numbers before i*: tvg & ~meet  (ops/after-op excluded)
            vnum = work.tile([P, S], F32, tag="vnum")
            nc.vector.scalar_tensor_tensor(
                vnum[:], meet[:], 1.0, tvg[:], op0=OP.not_equal, op1=OP.mult)

            # cpre[j] = inclusive prefix count of vnum
            cpre = work.tile([P, S], F32, tag="cpre")
            nc.vector.tensor_tensor_scan(
                cpre[:], vnum[:], vnum[:], 0.0, op0=OP.add, op1=OP.max)
            total = cpre[:, S - 1 : S]

            tm1 = sm.tile([P, 1], F32, tag=f"tm1_{t}", name=f"tm1_{t}")
            nc.vector.tensor_single_scalar(tm1[:], total, -1.0, op=OP.add)
            cnt1 = cnt1_a[:, t : t + 1]
            nc.vector.tensor_single_scalar(cnt1, total, 0.5, op=OP.is_gt)
            cnt2 = cnt2_a[:, t : t + 1]
            nc.vector.tensor_single_scalar(cnt2, total, 1.5, op=OP.is_gt)
            fire = fire_a[:, t : t + 1]
            nc.vector.tensor_mul(fire, met, cnt2)

            # last & second-to-last number one-hots
            ohp0 = keep.tile([P, S], F32, tag="ohp0")
            nc.vector.scalar_tensor_tensor(
                ohp0[:], cpre[:], total, vnum[:], op0=OP.is_equal, op1=OP.mult)
            ohp1 = keep.tile([P, S], F32, tag="ohp1")
            nc.vector.scalar_tensor_tensor(
                ohp1[:], cpre[:], tm1[:], vnum[:], op0=OP.is_equal, op1=OP.mult)

            # masked-sum gathers (exact: at most one nonzero term)
            def gather_dve(dst_col, src_ap, mask_ap):
                scr = gsc.tile([P, S], F32, tag="gscr", name="gscr")
                nc.vector.scalar_tensor_tensor(
                    scr[:], src_ap, 0.0, mask_ap, op0=OP.bypass, op1=OP.mult,
                    accum_out=dst_col)

            def gather_act(dst_col, src_ap, mask_ap):
                scr = gsc.tile([P, S], F32, tag="gscp", name="gscp")
                nc.gpsimd.tensor_mul(scr[:], src_ap, mask_ap)
                scr2 = gsc.tile([P, S], F32, tag="gscq", name="gscq")
                nc.scalar.activation(scr2[:], scr[:], ACTF.Copy, accum_out=dst_col)

            gather_dve(h0_a[:, t : t + 1], tdt[:], ohp0[:])
            gather_act(h1_a[:, t : t + 1], tdt[:], ohp1[:])
            gather_dve(vmax_a[:, t : t + 1], m7[:], ohi[:])
            for c, fn in zip(range(2, 7),
                             [gather_dve, gather_dve, gather_act, gather_act,
                              gather_dve]):
                fn(vc_a[c - 2][:, t : t + 1], opv[:, :, c], ohi[:])

            tv_tiles.append(tvt)
            td_tiles.append(tdt)
            ohp0_tiles.append(ohp0)
            ohp1_tiles.append(ohp1)
            ohi_tiles.append(ohi)

        # =========================== phase B ===========================
        radd = sm.tile([P, NT], F32)
        nc.vector.tensor_add(radd[:], h1_a[:], h0_a[:])
        rsub = sm.tile([P, NT], F32)
        nc.vector.tensor_sub(rsub[:], h1_a[:], h0_a[:])
        rmul = sm.tile([P, NT], F32)
        nc.vector.tensor_mul(rmul[:], h1_a[:], h0_a[:])
        den = sm.tile([P, NT], F32)
        nc.vector.tensor_scalar_add(den[:], h0_a[:], 1e-7)
        rec = sm.tile([P, NT], F32)
        nc.vector.reciprocal(rec[:], den[:])
        rdiv = sm.tile([P, NT], F32)
        nc.vector.tensor_mul(rdiv[:], h1_a[:], rec[:])
        base = sm.tile([P, NT], F32)
        nc.vector.tensor_scalar_max(base[:], h1_a[:], 1e-7)
        lg = sm.tile([P, NT], F32)
        nc.scalar.activation(lg[:], base[:], ACTF.Ln)
        pm = sm.tile([P, NT], F32)
        nc.vector.tensor_mul(pm[:], lg[:], h0_a[:])
        rpow = sm.tile([P, NT], F32)
        nc.scalar.activation(rpow[:], pm[:], ACTF.Exp)

        r_a = sm.tile([P, NT], F32)
        acc = sm.tile([P, NT], F32, tag="racc")
        ohc = sm.tile([P, NT], F32, tag="ohc")
        first = True
        for vc, res in zip(vc_a, [radd, rsub, rmul, rdiv, rpow]):
            nc.vector.tensor_tensor(ohc[:], vc[:], vmax_a[:], op=OP.is_equal)
            if first:
                nc.vector.tensor_mul(r_a[:], ohc[:], res[:])
                first = False
            else:
                nc.vector.tensor_mul(acc[:], ohc[:], res[:])
                nc.vector.tensor_add(r_a[:], r_a[:], acc[:])

        # iv = fire + (1-met)*g ; ifu = (1-met)*g*cnt1*(1-cnt2) ; iff = (1-f)*ifu + f
        nmet = sm.tile([P, NT], F32)
        nc.vector.tensor_scalar(nmet[:], met_a[:], -1.0, 1.0, op0=OP.mult, op1=OP.add)
        nmg = sm.tile([P, NT], F32)
        nc.vector.tensor_mul(nmg[:], nmet[:], val_t[:])
        iv_a = sm.tile([P, NT], F32)
        nc.vector.tensor_add(iv_a[:], fire_a[:], nmg[:])
        ncnt2 = sm.tile([P, NT], F32)
        nc.vector.tensor_scalar(ncnt2[:], cnt2_a[:], -1.0, 1.0, op0=OP.mult, op1=OP.add)
        q1 = sm.tile([P, NT], F32)
        nc.vector.tensor_mul(q1[:], nmg[:], cnt1_a[:])
        ifu = sm.tile([P, NT], F32)
        nc.vector.tensor_mul(ifu[:], q1[:], ncnt2[:])
        q2 = sm.tile([P, NT], F32)
        nc.vector.tensor_mul(q2[:], omf[:], ifu[:])
        iff_a = sm.tile([P, NT], F32)
        nc.vector.tensor_add(iff_a[:], q2[:], fin_t[:])

        nc.sync.dma_start(iff_d.ap().rearrange("(t p) -> p t", p=P), iff_a[:])
        nc.sync.dma_start(ivo_d.ap().rearrange("(t p) -> p t", p=P), iv_a[:])

        # =========================== phase C ===========================
        for t in range(NT):
            rows = slice(t * P, (t + 1) * P)
            tvt, tdt = tv_tiles[t], td_tiles[t]
            ohp0, ohp1, ohi = ohp0_tiles[t], ohp1_tiles[t], ohi_tiles[t]
            fire = fire_a[:, t : t + 1]
            r_t = r_a[:, t : t + 1]

            # td[p0] <- r where fire
            w0 = work.tile([P, S], mybir.dt.uint32, tag="w0")
            nc.vector.tensor_scalar(w0[:], ohp0[:], fire, None, op0=OP.mult)
            rb = work.tile([P, S], F32, tag="rb")
            nc.scalar.activation(rb[:], zero_bc[:], ACTF.Identity, bias=r_t, scale=1.0)
            nc.vector.copy_predicated(tdt[:], w0[:], rb[:])

            # tv[p1] <- 0, tv[i*] <- 0 where fire
            u1 = work.tile([P, S], F32, tag="u1")
            nc.any.tensor_add(u1[:], ohp1[:], ohi[:])
            w1 = work.tile([P, S], F32, tag="w1")
            nc.vector.tensor_scalar(w1[:], u1[:], fire, None, op0=OP.mult)
            nc.vector.scalar_tensor_tensor(
                tvt[:], w1[:], 1.0, tvt[:], op0=OP.not_equal, op1=OP.mult)

            nc.sync.dma_start(tvo_d[rows, :], tvt[:])
            nc.sync.dma_start(tdo_d[rows, :], tdt[:])

    nc.compile()
    return nc


_NC_CACHE = None


def _get_nc():
    global _NC_CACHE
    if _NC_CACHE is None:
        _NC_CACHE = _build_nc()
    return _NC_CACHE


def _pad_op8(op):
    """[B, S, 7] f32 -> [B, S*8] with channel 7 = -inf (layout-only)."""
    op8 = np.empty((B, S, C8), np.float32)
    op8[:, :, :NOPS] = op
    op8[:, :, NOPS] = -np.inf
    return op8.reshape(B, S * C8)


def kernel(trans_valid, trans_dense, trans_op, if_finished, if_valid, start_pos):
    tv = np.ascontiguousarray(np.asarray(trans_valid, np.float32))
    td = np.ascontiguousarray(np.asarray(trans_dense, np.float32))
    op = _pad_op8(np.asarray(trans_op, np.float32))
    fin = np.ascontiguousarray(np.asarray(if_finished, np.float32))
    val = np.ascontiguousarray(np.asarray(if_valid, np.float32))
    sp = int(start_pos)
    act = (np.arange(S) >= sp).astype(np.float32)

    nc = _get_nc()
    in_maps = []
    for c in range(NCORES):
        rows = slice(c * BS, (c + 1) * BS)
        in_maps.append({
            "tv": tv[rows], "td": td[rows], "op": op[rows],
            "fin": fin[rows], "val": val[rows], "act": act,
        })
    res = run_bass_kernel_spmd(nc, in_maps, core_ids=list(range(NCORES)))
    outs = res.results

    tv_out = np.concatenate([outs[c]["tv_out"] for c in range(NCORES)], axis=0)
    td_out = np.concatenate([outs[c]["td_out"] for c in range(NCORES)], axis=0)
    iff = np.concatenate([outs[c]["iff_out"] for c in range(NCORES)], axis=0)
    iv = np.concatenate([outs[c]["iv_out"] for c in range(NCORES)], axis=0)

    return tv_out, td_out, np.asarray(trans_op, np.float32), iff, iv


# revision 10
# speedup vs baseline: 1.1241x; 1.1241x over previous
"""Trainium2 Bass kernel for nn_ArthDenseCalcToDenseBlock.

The reference is a 256-step sequential scan per batch row, but the state
machine freezes at the first valid operator token (the `meet` gate), so the
whole scan collapses to closed-form masked reductions along the sequence
axis, computed per row with DVE prefix-scan instructions:

  meet[j] = running-max of (valid-op mask)        -> first-op one-hot, met
  csuf[j] = reverse running-sum of (number mask)  -> last / 2nd-last number
                                                     one-hots via == 1 / == 2
  h0,h1 and the operator channel values are gathered with masked-sum
  accumulations; one predicated scatter writes the result back.

Data parallel over batch: 4096 rows -> 8 cores x 512 rows -> 2 halves of
[128, 2x256] merged tiles per core. Mask tensors are bf16 (all values are
0/1 or small integers, exactly representable); trans_op values and
trans_dense stay f32 so argmax/select semantics match the reference
bit-exactly. Work is spread across DVE / GpSimd / ACT.
"""

from contextlib import ExitStack

import numpy as np

import concourse.bacc as bacc
import concourse.mybir as mybir
import concourse.tile as tile
from concourse.bass_utils import run_bass_kernel_spmd

F32 = mybir.dt.float32
BF16 = mybir.dt.bfloat16
U8 = mybir.dt.uint8
OP = mybir.AluOpType
ACTF = mybir.ActivationFunctionType
AX = mybir.AxisListType

B, S, NOPS = 4096, 256, 7
NCORES = 8
BS = B // NCORES          # rows per core (512)
P = 128                   # partitions
NT = BS // P              # row-tiles per core (4)
NH = 2                    # halves per core
TPH = NT // NH            # row-tiles per half (2)
W = TPH * S               # free width of a merged half (512)


def _build_nc():
    nc = bacc.Bacc("TRN2", target_bir_lowering=False, debug=False)

    tv_d = nc.dram_tensor("tv", [BS, S], F32, kind="ExternalInput")
    td_d = nc.dram_tensor("td", [BS, S], F32, kind="ExternalInput")
    op_d = nc.dram_tensor("op", [BS, S * NOPS], F32, kind="ExternalInput")
    fv_d = nc.dram_tensor("fv", [P, 2 * NT], F32, kind="ExternalInput")
    act_d = nc.dram_tensor("act2", [W], F32, kind="ExternalInput")

    tvo_d = nc.dram_tensor("tv_out", [BS, S], F32, kind="ExternalOutput")
    tdo_d = nc.dram_tensor("td_out", [BS, S], F32, kind="ExternalOutput")
    io_d = nc.dram_tensor("iffiv", [P * 2 * NT], F32, kind="ExternalOutput")

    with tile.TileContext(nc) as tc, ExitStack() as ctx:
        cpool = ctx.enter_context(tc.tile_pool(name="consts", bufs=1))
        io_pool = ctx.enter_context(tc.tile_pool(name="io", bufs=NH + 1))
        op_pool = ctx.enter_context(tc.tile_pool(name="op", bufs=2))
        work = ctx.enter_context(tc.tile_pool(name="work", bufs=2))
        gsc = ctx.enter_context(tc.tile_pool(name="gsc", bufs=4))
        sm = ctx.enter_context(tc.tile_pool(name="small", bufs=1))

        # ---- constants: act mask (already tiled x2 on host) -> bf16 bcast
        crow = cpool.tile([P, W], F32)
        nc.sync.dma_start(crow[0:1, :], act_d.ap().rearrange("(o s) -> o s", o=1))
        actf = cpool.tile([P, W], F32)
        nc.gpsimd.partition_broadcast(actf[:], crow[0:1, :])
        act_bc = cpool.tile([P, W], BF16)
        nc.vector.tensor_copy(act_bc[:], actf[:])
        zero_bc = cpool.tile([P, S], F32)
        nc.vector.memset(zero_bc[:], 0.0)

        # ---- per-row gates [P, NT] (f cols 0..NT-1, g cols NT..2NT-1)
        fv = sm.tile([P, 2 * NT], F32)
        nc.sync.dma_start(fv[:], fv_d[:, :])
        fin_t = fv[:, 0:NT]
        val_t = fv[:, NT : 2 * NT]
        omf = sm.tile([P, NT], F32)   # 1 - f
        nc.vector.tensor_scalar(omf[:], fin_t, -1.0, 1.0, op0=OP.mult, op1=OP.add)
        gate = sm.tile([P, NT], F32)  # (1 - f) * g
        nc.vector.tensor_mul(gate[:], omf[:], val_t)

        iffiv = sm.tile([P, 2 * NT], F32)

        for h in range(NH):
            rows = slice(h * TPH * P, (h + 1) * TPH * P)
            tvt = io_pool.tile([P, W], F32, tag="tvt", name=f"tvt{h}")
            tdt = io_pool.tile([P, W], F32, tag="tdt", name=f"tdt{h}")
            opt = op_pool.tile([P, TPH * S * NOPS], F32, tag="opt", name=f"opt{h}")
            # op first: the longest pole
            nc.sync.dma_start(
                opt[:].rearrange("p (t q) -> p t q", t=TPH),
                op_d[rows, :].rearrange("(t p) q -> p t q", p=P))
            nc.sync.dma_start(
                tvt[:].rearrange("p (t s) -> p t s", t=TPH),
                tv_d[rows, :].rearrange("(t p) s -> p t s", p=P))
            nc.sync.dma_start(
                tdt[:].rearrange("p (t s) -> p t s", t=TPH),
                td_d[rows, :].rearrange("(t p) s -> p t s", p=P))

            opv = opt[:].rearrange("p (t s c) -> p t s c", t=TPH, c=NOPS)
            ch = [opv[:, :, :, c] for c in range(NOPS)]  # [P,TPH,S] each

            def T2(t):
                return t[:].rearrange("p (t s) -> p t s", t=TPH)

            # channel max over 1..6 via TT tree, spread across engines
            a1 = work.tile([P, W], F32, tag="a1", name=f"a1_{h}")
            nc.vector.tensor_tensor(T2(a1), ch[1], ch[2], op=OP.max)
            a2 = work.tile([P, W], F32, tag="a2", name=f"a2_{h}")
            nc.vector.tensor_tensor(T2(a2), ch[3], ch[4], op=OP.max)
            a3 = work.tile([P, W], F32, tag="a3", name=f"a3_{h}")
            nc.any.tensor_tensor(T2(a3), ch[5], ch[6], op=OP.max)
            b1 = work.tile([P, W], F32, tag="b1", name=f"b1_{h}")
            nc.any.tensor_tensor(b1[:], a1[:], a2[:], op=OP.max)
            m6 = work.tile([P, W], F32, tag="m6", name=f"m6_{h}")
            nc.vector.tensor_tensor(m6[:], b1[:], a3[:], op=OP.max)
            is_op = work.tile([P, W], BF16, tag="isop", name=f"isop_{h}")
            nc.any.tensor_tensor(T2(is_op), T2(m6), ch[0], op=OP.is_gt)

            # valid-token masks (f/g gate folded into per-row scalars later)
            tvb = work.tile([P, W], BF16, tag="tvb", name=f"tvb_{h}")
            nc.vector.tensor_copy(tvb[:], tvt[:])
            tvga = work.tile([P, W], BF16, tag="tvga", name=f"tvga_{h}")
            nc.gpsimd.tensor_mul(tvga[:], tvb[:], act_bc[:])
            vop = work.tile([P, W], BF16, tag="vop", name=f"vop_{h}")
            nc.gpsimd.tensor_mul(vop[:], tvga[:], is_op[:])

            # meet[j] = running max of vop (per 256-tile)
            meet = work.tile([P, W], BF16, tag="meet", name=f"meet_{h}")
            for t in range(TPH):
                ts = slice(t * S, (t + 1) * S)
                nc.vector.tensor_tensor_scan(
                    meet[:][:, ts], vop[:][:, ts], vop[:][:, ts], 0.0,
                    op0=OP.max, op1=OP.max)

            # first-op one-hot: vop & ~meet_exclusive (shift by 1)
            ohi = work.tile([P, W], BF16, tag="ohi", name=f"ohi_{h}")
            nc.vector.scalar_tensor_tensor(
                ohi[:][:, 1:W], meet[:][:, 0 : W - 1], 1.0, vop[:][:, 1:W],
                op0=OP.not_equal, op1=OP.mult)
            nc.vector.tensor_copy(ohi[:][:, 0::S], vop[:][:, 0::S])

            # numbers before i*: tvga & ~meet
            vnum = work.tile([P, W], BF16, tag="vnum", name=f"vnum_{h}")
            nc.vector.scalar_tensor_tensor(
                vnum[:], meet[:], 1.0, tvga[:], op0=OP.not_equal, op1=OP.mult)

            # csuf[j] = inclusive suffix count of vnum (reverse scan per tile)
            csuf = work.tile([P, W], BF16, tag="csuf", name=f"csuf_{h}")
            for t in range(TPH):
                ts = slice(t * S, (t + 1) * S)
                vr = vnum[:][:, ts][:, ::-1]
                nc.vector.tensor_tensor_scan(
                    csuf[:][:, ts][:, ::-1], vr, vr, 0.0, op0=OP.add, op1=OP.max)

            # last & second-to-last number one-hots (constant compares)
            ohp0 = work.tile([P, W], BF16, tag="ohp0", name=f"ohp0_{h}")
            nc.vector.scalar_tensor_tensor(
                ohp0[:], csuf[:], 1.0, vnum[:], op0=OP.is_equal, op1=OP.mult)
            ohp1 = work.tile([P, W], BF16, tag="ohp1", name=f"ohp1_{h}")
            nc.vector.scalar_tensor_tensor(
                ohp1[:], csuf[:], 2.0, vnum[:], op0=OP.is_equal, op1=OP.mult)

            # per-row scalars for this half: [P, TPH] strided views
            met2 = meet[:][:, S - 1 :: S]       # [P, TPH] bf16
            total2 = csuf[:][:, 0::S]           # [P, TPH] bf16
            gate2 = gate[:, h * TPH : (h + 1) * TPH]

            cnt1 = sm.tile([P, TPH], F32, name=f"cnt1_{h}")
            nc.gpsimd.tensor_scalar(cnt1[:], total2, 0.5, None, op0=OP.is_gt)
            cnt2 = sm.tile([P, TPH], F32, name=f"cnt2_{h}")
            nc.gpsimd.tensor_scalar(cnt2[:], total2, 1.5, None, op0=OP.is_gt)
            metg = sm.tile([P, TPH], F32, name=f"metg_{h}")
            nc.gpsimd.tensor_mul(metg[:], met2, gate2)
            fire = sm.tile([P, TPH], F32, name=f"fire_{h}")
            nc.gpsimd.tensor_mul(fire[:], metg[:], cnt2[:])
            fireb = sm.tile([P, TPH], BF16, name=f"fireb_{h}")
            nc.gpsimd.tensor_copy(fireb[:], fire[:])

            # masked-sum gathers (exact: at most one nonzero term)
            h0_2 = sm.tile([P, TPH], F32, name=f"h0_{h}")
            h1_2 = sm.tile([P, TPH], F32, name=f"h1_{h}")
            vmax_2 = sm.tile([P, TPH], F32, name=f"vmax_{h}")
            vc_2 = [sm.tile([P, TPH], F32, name=f"vc{c}_{h}") for c in range(2, 7)]

            def gather_dve(dst_col, src_ap, mask_ap):
                scr = gsc.tile([P, S], F32, tag="gscr", name="gscr")
                nc.vector.scalar_tensor_tensor(
                    scr[:], src_ap, 0.0, mask_ap, op0=OP.bypass, op1=OP.mult,
                    accum_out=dst_col)

            def gather_act(dst_col, src_ap, mask_ap):
                scr = gsc.tile([P, S], F32, tag="gscp", name="gscp")
                nc.gpsimd.tensor_mul(scr[:], src_ap, mask_ap)
                scr2 = gsc.tile([P, S], F32, tag="gscq", name="gscq")
                nc.scalar.activation(scr2[:], scr[:], ACTF.Copy,
                                     accum_out=dst_col)

            for t in range(TPH):
                ts = slice(t * S, (t + 1) * S)
                col = slice(t, t + 1)
                tds = tdt[:][:, ts]
                ohp0s, ohp1s, ohis = ohp0[:][:, ts], ohp1[:][:, ts], ohi[:][:, ts]
                gather_dve(h0_2[:, col], tds, ohp0s)
                gather_dve(h1_2[:, col], tds, ohp1s)
                gather_act(vmax_2[:, col], m6[:][:, ts], ohis)
                chs = [opv[:, t, :, c] for c in range(NOPS)]
                gather_dve(vc_2[0][:, col], chs[2], ohis)
                gather_dve(vc_2[1][:, col], chs[3], ohis)
                gather_act(vc_2[2][:, col], chs[4], ohis)
                gather_act(vc_2[3][:, col], chs[5], ohis)
                gather_dve(vc_2[4][:, col], chs[6], ohis)

            # ---- per-half small math ([P,TPH])
            radd = sm.tile([P, TPH], F32, name=f"radd_{h}")
            nc.gpsimd.tensor_add(radd[:], h1_2[:], h0_2[:])
            rsub = sm.tile([P, TPH], F32, name=f"rsub_{h}")
            nc.gpsimd.tensor_sub(rsub[:], h1_2[:], h0_2[:])
            rmul = sm.tile([P, TPH], F32, name=f"rmul_{h}")
            nc.gpsimd.tensor_mul(rmul[:], h1_2[:], h0_2[:])
            den = sm.tile([P, TPH], F32, name=f"den_{h}")
            nc.gpsimd.tensor_scalar_add(den[:], h0_2[:], 1e-7)
            rec = sm.tile([P, TPH], F32, name=f"rec_{h}")
            nc.vector.reciprocal(rec[:], den[:])
            rdiv = sm.tile([P, TPH], F32, name=f"rdiv_{h}")
            nc.gpsimd.tensor_mul(rdiv[:], h1_2[:], rec[:])
            base = sm.tile([P, TPH], F32, name=f"base_{h}")
            nc.gpsimd.tensor_scalar_max(base[:], h1_2[:], 1e-7)
            lg = sm.tile([P, TPH], F32, name=f"lg_{h}")
            nc.scalar.activation(lg[:], base[:], ACTF.Ln)
            pm = sm.tile([P, TPH], F32, name=f"pm_{h}")
            nc.gpsimd.tensor_mul(pm[:], lg[:], h0_2[:])
            rpow = sm.tile([P, TPH], F32, name=f"rpow_{h}")
            nc.scalar.activation(rpow[:], pm[:], ACTF.Exp)

            # r = result selected by the argmax channel at i*
            r_2 = sm.tile([P, TPH], F32, name=f"r_{h}")
            nc.vector.memset(r_2[:], 0.0)
            ohc = sm.tile([P, TPH], U8, name=f"ohc{h}", bufs=2)
            for vc, res in zip(vc_2, [radd, rsub, rmul, rdiv, rpow]):
                nc.vector.tensor_tensor(ohc[:], vc[:], vmax_2[:], op=OP.is_equal)
                nc.vector.copy_predicated(r_2[:], ohc[:], res[:])

            # iv = fire + (1-met_g)*g ; iff = (1-f)*(1-met_g)*g*cnt1*(1-cnt2)+f
            nmet = sm.tile([P, TPH], F32, name=f"nmet_{h}")
            nc.gpsimd.tensor_scalar(nmet[:], metg[:], -1.0, 1.0,
                                    op0=OP.mult, op1=OP.add)
            g2 = val_t[:, h * TPH : (h + 1) * TPH]
            nmg = sm.tile([P, TPH], F32, name=f"nmg_{h}")
            nc.gpsimd.tensor_mul(nmg[:], nmet[:], g2)
            nc.gpsimd.tensor_add(iffiv[:, NT + h * TPH : NT + (h + 1) * TPH],
                                 fire[:], nmg[:])
            ncnt2 = sm.tile([P, TPH], F32, name=f"ncnt2_{h}")
            nc.gpsimd.tensor_scalar(ncnt2[:], cnt2[:], -1.0, 1.0,
                                    op0=OP.mult, op1=OP.add)
            q1 = sm.tile([P, TPH], F32, name=f"q1_{h}")
            nc.gpsimd.tensor_mul(q1[:], nmg[:], cnt1[:])
            ifu = sm.tile([P, TPH], F32, name=f"ifu_{h}")
            nc.gpsimd.tensor_mul(ifu[:], q1[:], ncnt2[:])
            q2 = sm.tile([P, TPH], F32, name=f"q2_{h}")
            nc.gpsimd.tensor_mul(q2[:], omf[:, h * TPH : (h + 1) * TPH], ifu[:])
            nc.gpsimd.tensor_add(iffiv[:, h * TPH : (h + 1) * TPH],
                                 q2[:], fin_t[:, h * TPH : (h + 1) * TPH])

            # ---- scatter
            fire_bc = fireb[:].rearrange("p (t o) -> p t o", o=1).to_broadcast(
                [P, TPH, S])

            # td[p0] <- r where fire
            w0 = work.tile([P, W], U8, tag="w0", name=f"w0_{h}")
            nc.vector.tensor_tensor(T2(w0), T2(ohp0), fire_bc, op=OP.mult)
            rb = work.tile([P, W], F32, tag="rb", name=f"rb_{h}")
            for t in range(TPH):
                nc.scalar.activation(rb[:][:, t * S : (t + 1) * S], zero_bc[:],
                                     ACTF.Identity, bias=r_2[:, t : t + 1],
                                     scale=1.0)
            nc.vector.copy_predicated(tdt[:], w0[:], rb[:])

            # tv[p1] <- 0, tv[i*] <- 0 where fire
            u1 = work.tile([P, W], BF16, tag="u1", name=f"u1_{h}")
            nc.gpsimd.tensor_add(u1[:], ohp1[:], ohi[:])
            w1 = work.tile([P, W], BF16, tag="w1", name=f"w1_{h}")
            nc.vector.tensor_tensor(T2(w1), T2(u1), fire_bc, op=OP.mult)
            nc.vector.scalar_tensor_tensor(
                tvt[:], w1[:], 1.0, tvt[:], op0=OP.not_equal, op1=OP.mult)

            nc.scalar.dma_start(
                tvo_d[rows, :].rearrange("(t p) s -> p t s", p=P),
                tvt[:].rearrange("p (t s) -> p t s", t=TPH))
            nc.scalar.dma_start(
                tdo_d[rows, :].rearrange("(t p) s -> p t s", p=P),
                tdt[:].rearrange("p (t s) -> p t s", t=TPH))

        nc.scalar.dma_start(io_d.ap().rearrange("(p q) -> p q", p=P), iffiv[:])

    nc.compile()
    return nc


_NC_CACHE = None


def _get_nc():
    global _NC_CACHE
    if _NC_CACHE is None:
        _NC_CACHE = _build_nc()
    return _NC_CACHE


def _make_in_maps(trans_valid, trans_dense, trans_op, if_finished, if_valid,
                  start_pos):
    tv = np.ascontiguousarray(np.asarray(trans_valid, np.float32))
    td = np.ascontiguousarray(np.asarray(trans_dense, np.float32))
    op = np.ascontiguousarray(np.asarray(trans_op, np.float32)).reshape(B, S * NOPS)
    fin = np.asarray(if_finished, np.float32)
    val = np.asarray(if_valid, np.float32)
    sp = int(start_pos)
    act2 = np.ascontiguousarray(
        np.tile((np.arange(S) >= sp).astype(np.float32), TPH))
    in_maps = []
    for c in range(NCORES):
        rows = slice(c * BS, (c + 1) * BS)
        # fv[p, t] = fin[t*128+p] ; fv[p, NT+t] = val[t*128+p]
        fvc = np.concatenate(
            [fin[rows].reshape(NT, P).T, val[rows].reshape(NT, P).T], axis=1)
        in_maps.append({
            "tv": tv[rows], "td": td[rows], "op": op[rows],
            "fv": np.ascontiguousarray(fvc), "act2": act2,
        })
    return in_maps


def _unpack_outs(outs, trans_op):
    tv_out = np.concatenate([outs[c]["tv_out"] for c in range(NCORES)], axis=0)
    td_out = np.concatenate([outs[c]["td_out"] for c in range(NCORES)], axis=0)
    iff = np.empty(B, np.float32)
    iv = np.empty(B, np.float32)
    for c in range(NCORES):
        arr = outs[c]["iffiv"].reshape(P, 2 * NT)
        rows = slice(c * BS, (c + 1) * BS)
        iff[rows] = arr[:, 0:NT].T.reshape(BS)
        iv[rows] = arr[:, NT : 2 * NT].T.reshape(BS)
    return tv_out, td_out, np.asarray(trans_op, np.float32), iff, iv


def kernel(trans_valid, trans_dense, trans_op, if_finished, if_valid, start_pos):
    nc = _get_nc()
    in_maps = _make_in_maps(trans_valid, trans_dense, trans_op, if_finished,
                            if_valid, start_pos)
    res = run_bass_kernel_spmd(nc, in_maps, core_ids=list(range(NCORES)))
    outs = res.results

    return _unpack_outs(outs, trans_op)


# revision 11
# speedup vs baseline: 1.4328x; 1.2746x over previous
"""Trainium2 Bass kernel for nn_ArthDenseCalcToDenseBlock.

The reference is a 256-step sequential scan per batch row, but the state
machine freezes at the first valid operator token (the `meet` gate), so the
whole scan collapses to closed-form masked reductions along the sequence
axis, computed per row with DVE prefix-scan instructions:

  mpre[j] = running-max of (valid-op mask)        -> first-op one-hot, met
  csuf[j] = reverse running-sum of (number mask)  -> last / 2nd-last number
                                                     one-hots via == 1 / == 2
  h0,h1 and the operator channel values are gathered with masked-sum
  accumulations; one predicated scatter writes the result back.

Data parallel over batch: 4096 rows -> 8 cores x 512 rows -> 2 halves of
[128, 2x256] merged tiles per core. trans_op is host-relayouted into 7
contiguous channel planes so every channel op is a contiguous 2D access.
Mask tensors are bf16 (0/1 and small counts are exact); trans_op values and
trans_dense stay f32 so argmax/select semantics match the reference
bit-exactly. Work is spread across DVE / GpSimd / ACT.
"""

from contextlib import ExitStack

import numpy as np

import concourse.bacc as bacc
import concourse.mybir as mybir
import concourse.tile as tile
from concourse.bass_utils import run_bass_kernel_spmd

F32 = mybir.dt.float32
BF16 = mybir.dt.bfloat16
U8 = mybir.dt.uint8
OP = mybir.AluOpType
ACTF = mybir.ActivationFunctionType

B, S, NOPS = 4096, 256, 7
NCORES = 8
BS = B // NCORES          # rows per core (512)
P = 128                   # partitions
NT = BS // P              # row-tiles per core (4)
NH = 2                    # halves per core
TPH = NT // NH            # row-tiles per half (2)
W = TPH * S               # free width of a merged half (512)


def _build_nc(sp_zero: bool):
    nc = bacc.Bacc("TRN2", target_bir_lowering=False, debug=False)

    tv_d = nc.dram_tensor("tv", [BS, S], F32, kind="ExternalInput")
    td_d = nc.dram_tensor("td", [BS, S], F32, kind="ExternalInput")
    # channel planes: op[c, row, s]
    op_d = nc.dram_tensor("op", [NOPS, BS, S], F32, kind="ExternalInput")
    fv_d = nc.dram_tensor("fv", [P, 2 * NT], F32, kind="ExternalInput")
    act_d = nc.dram_tensor("act2", [W], F32, kind="ExternalInput")

    tvo_d = nc.dram_tensor("tv_out", [BS, S], F32, kind="ExternalOutput")
    tdo_d = nc.dram_tensor("td_out", [BS, S], F32, kind="ExternalOutput")
    io_d = nc.dram_tensor("iffiv", [P * 2 * NT], F32, kind="ExternalOutput")

    with tile.TileContext(nc) as tc, ExitStack() as ctx:
        cpool = ctx.enter_context(tc.tile_pool(name="consts", bufs=1))
        io_pool = ctx.enter_context(tc.tile_pool(name="io", bufs=NH + 1))
        op_pool = ctx.enter_context(tc.tile_pool(name="op", bufs=2))
        work = ctx.enter_context(tc.tile_pool(name="work", bufs=2))
        gsc = ctx.enter_context(tc.tile_pool(name="gsc", bufs=6))
        sm = ctx.enter_context(tc.tile_pool(name="small", bufs=1))

        if not sp_zero:
            crow = cpool.tile([P, W], F32)
            nc.sync.dma_start(crow[0:1, :],
                              act_d.ap().rearrange("(o s) -> o s", o=1))
            actf = cpool.tile([P, W], F32)
            nc.gpsimd.partition_broadcast(actf[:], crow[0:1, :])
            act_bc = cpool.tile([P, W], BF16)
            nc.vector.tensor_copy(act_bc[:], actf[:])
        zero_bc = cpool.tile([P, S], F32)
        nc.vector.memset(zero_bc[:], 0.0)

        # ---- per-row gates [P, NT] (f cols 0..NT-1, g cols NT..2NT-1)
        fv = sm.tile([P, 2 * NT], F32)
        nc.sync.dma_start(fv[:], fv_d[:, :])
        fin_t = fv[:, 0:NT]
        val_t = fv[:, NT : 2 * NT]
        omf = sm.tile([P, NT], F32)   # 1 - f
        nc.gpsimd.tensor_scalar(omf[:], fin_t, -1.0, 1.0, op0=OP.mult, op1=OP.add)
        gate = sm.tile([P, NT], F32)  # (1 - f) * g
        nc.gpsimd.tensor_mul(gate[:], omf[:], val_t)

        iffiv = sm.tile([P, 2 * NT], F32)
        # batched per-core scalars (columns = row-tile index 0..NT-1)
        h0_a = sm.tile([P, NT], F32)
        h1_a = sm.tile([P, NT], F32)
        vmax_a = sm.tile([P, NT], F32)
        vc_a = [sm.tile([P, NT], F32, name=f"vc{c}") for c in range(2, 7)]
        fire_a = sm.tile([P, NT], F32)
        r_a = sm.tile([P, NT], F32)

        half_state = []

        for h in range(NH):
            rows = slice(h * TPH * P, (h + 1) * TPH * P)
            tvt = io_pool.tile([P, W], F32, tag="tvt", name=f"tvt{h}")
            tdt = io_pool.tile([P, W], F32, tag="tdt", name=f"tdt{h}")
            chs = []
            for c in range(NOPS):
                cht = op_pool.tile([P, W], F32, tag=f"ch{c}", name=f"ch{c}_{h}")
                nc.sync.dma_start(
                    cht[:].rearrange("p (t s) -> p t s", t=TPH),
                    op_d[c, rows, :].rearrange("(t p) s -> p t s", p=P))
                chs.append(cht)
            nc.sync.dma_start(
                tvt[:].rearrange("p (t s) -> p t s", t=TPH),
                tv_d[rows, :].rearrange("(t p) s -> p t s", p=P))
            nc.sync.dma_start(
                tdt[:].rearrange("p (t s) -> p t s", t=TPH),
                td_d[rows, :].rearrange("(t p) s -> p t s", p=P))

            # channel max over 1..6 via TT tree (DVE/ACT; Pool has no max)
            a1 = work.tile([P, W], F32, tag="a1", name=f"a1_{h}")
            nc.vector.tensor_max(a1[:], chs[1][:], chs[2][:])
            a2 = work.tile([P, W], F32, tag="a2", name=f"a2_{h}")
            nc.vector.tensor_max(a2[:], chs[3][:], chs[4][:])
            a3 = work.tile([P, W], F32, tag="a3", name=f"a3_{h}")
            nc.any.tensor_max(a3[:], chs[5][:], chs[6][:])
            b1 = work.tile([P, W], F32, tag="b1", name=f"b1_{h}")
            nc.any.tensor_max(b1[:], a1[:], a2[:])
            m6 = work.tile([P, W], F32, tag="m6", name=f"m6_{h}")
            nc.vector.tensor_max(m6[:], b1[:], a3[:])
            is_op = work.tile([P, W], BF16, tag="isop", name=f"isop_{h}")
            nc.any.tensor_tensor(is_op[:], m6[:], chs[0][:], op=OP.is_gt)

            # valid-token mask (f/g gate folded into per-row scalars later)
            tvb = work.tile([P, W], BF16, tag="tvb", name=f"tvb_{h}")
            nc.vector.tensor_copy(tvb[:], tvt[:])
            if not sp_zero:
                tva = work.tile([P, W], BF16, tag="tva", name=f"tva_{h}")
                nc.vector.tensor_mul(tva[:], tvb[:], act_bc[:])
                tvb = tva
            vop = work.tile([P, W], BF16, tag="vop", name=f"vop_{h}")
            nc.vector.tensor_mul(vop[:], tvb[:], is_op[:])

            # mpre[j] = running max of vop (per 256-tile)
            mpre = work.tile([P, W], BF16, tag="mpre", name=f"mpre_{h}")
            for t in range(TPH):
                ts = slice(t * S, (t + 1) * S)
                nc.vector.tensor_tensor_scan(
                    mpre[:][:, ts], vop[:][:, ts], vop[:][:, ts], 0.0,
                    op0=OP.max, op1=OP.max)

            # nsh[j] = 1 if no valid op strictly before j (within tile)
            nsh = work.tile([P, W], BF16, tag="nsh", name=f"nsh_{h}")
            nc.vector.tensor_single_scalar(
                nsh[:][:, 1:W], mpre[:][:, 0 : W - 1], 1.0, op=OP.not_equal)
            nc.vector.memset(nsh[:][:, 0::S], 1.0)
            ohi = work.tile([P, W], BF16, tag="ohi", name=f"ohi_{h}")
            nc.vector.tensor_mul(ohi[:], nsh[:], vop[:])

            # numbers before i*: tvb & ~mpre
            nm = work.tile([P, W], BF16, tag="nm", name=f"nm_{h}")
            nc.vector.tensor_single_scalar(nm[:], mpre[:], 1.0, op=OP.not_equal)
            vnum = work.tile([P, W], BF16, tag="vnum", name=f"vnum_{h}")
            nc.vector.tensor_mul(vnum[:], nm[:], tvb[:])

            # csuf[j] = inclusive suffix count of vnum (reverse scan per tile)
            csuf = work.tile([P, W], BF16, tag="csuf", name=f"csuf_{h}")
            for t in range(TPH):
                ts = slice(t * S, (t + 1) * S)
                vr = vnum[:][:, ts][:, ::-1]
                nc.vector.tensor_tensor_scan(
                    csuf[:][:, ts][:, ::-1], vr, vr, 0.0, op0=OP.add, op1=OP.max)

            # last & second-to-last number one-hots (constant compares)
            e0 = work.tile([P, W], BF16, tag="e0", name=f"e0_{h}")
            nc.vector.tensor_single_scalar(e0[:], csuf[:], 1.0, op=OP.is_equal)
            ohp0 = work.tile([P, W], BF16, tag="ohp0", name=f"ohp0_{h}")
            nc.vector.tensor_mul(ohp0[:], e0[:], vnum[:])
            e1 = work.tile([P, W], BF16, tag="e1", name=f"e1_{h}")
            nc.vector.tensor_single_scalar(e1[:], csuf[:], 2.0, op=OP.is_equal)
            ohp1 = work.tile([P, W], BF16, tag="ohp1", name=f"ohp1_{h}")
            nc.vector.tensor_mul(ohp1[:], e1[:], vnum[:])

            # per-row scalars for this half
            met2 = mpre[:][:, S - 1 :: S]       # [P, TPH] bf16
            total2 = csuf[:][:, 0::S]           # [P, TPH] bf16
            gate2 = gate[:, h * TPH : (h + 1) * TPH]
            acols = slice(h * TPH, (h + 1) * TPH)

            cnt1 = sm.tile([P, TPH], F32, name=f"cnt1_{h}")
            nc.gpsimd.tensor_scalar(cnt1[:], total2, 0.5, None, op0=OP.is_gt)
            cnt2 = sm.tile([P, TPH], F32, name=f"cnt2_{h}")
            nc.gpsimd.tensor_scalar(cnt2[:], total2, 1.5, None, op0=OP.is_gt)
            metg = sm.tile([P, TPH], F32, name=f"metg_{h}")
            nc.gpsimd.tensor_mul(metg[:], met2, gate2)
            fire2 = fire_a[:, acols]
            nc.gpsimd.tensor_mul(fire2, metg[:], cnt2[:])

            # masked-sum gathers (exact: at most one nonzero term)
            def gather_dve(dst_col, src_ap, mask_ap):
                scr = gsc.tile([P, S], F32, tag="gscr", name="gscr")
                nc.vector.scalar_tensor_tensor(
                    scr[:], src_ap, 0.0, mask_ap, op0=OP.bypass, op1=OP.mult,
                    accum_out=dst_col)

            def gather_act(dst_col, src_ap, mask_ap):
                scr = gsc.tile([P, S], F32, tag="gscp", name="gscp")
                nc.gpsimd.tensor_mul(scr[:], src_ap, mask_ap)
                scr2 = gsc.tile([P, S], F32, tag="gscq", name="gscq")
                nc.scalar.activation(scr2[:], scr[:], ACTF.Copy,
                                     accum_out=dst_col)

            for t in range(TPH):
                ts = slice(t * S, (t + 1) * S)
                col = slice(h * TPH + t, h * TPH + t + 1)
                tds = tdt[:][:, ts]
                ohp0s, ohp1s, ohis = ohp0[:][:, ts], ohp1[:][:, ts], ohi[:][:, ts]
                gather_dve(h0_a[:, col], tds, ohp0s)
                gather_dve(h1_a[:, col], tds, ohp1s)
                gather_act(vmax_a[:, col], m6[:][:, ts], ohis)
                gather_dve(vc_a[0][:, col], chs[2][:][:, ts], ohis)
                gather_dve(vc_a[1][:, col], chs[3][:][:, ts], ohis)
                gather_act(vc_a[2][:, col], chs[4][:][:, ts], ohis)
                gather_act(vc_a[3][:, col], chs[5][:][:, ts], ohis)
                gather_act(vc_a[4][:, col], chs[6][:][:, ts], ohis)

            # iv / iff (gps smalls)
            nmet = sm.tile([P, TPH], F32, name=f"nmet_{h}")
            nc.gpsimd.tensor_scalar(nmet[:], metg[:], -1.0, 1.0,
                                    op0=OP.mult, op1=OP.add)
            g2 = val_t[:, acols]
            nmg = sm.tile([P, TPH], F32, name=f"nmg_{h}")
            nc.gpsimd.tensor_mul(nmg[:], nmet[:], g2)
            nc.gpsimd.tensor_add(iffiv[:, NT + h * TPH : NT + (h + 1) * TPH],
                                 fire2, nmg[:])
            ncnt2 = sm.tile([P, TPH], F32, name=f"ncnt2_{h}")
            nc.gpsimd.tensor_scalar(ncnt2[:], cnt2[:], -1.0, 1.0,
                                    op0=OP.mult, op1=OP.add)
            q1 = sm.tile([P, TPH], F32, name=f"q1_{h}")
            nc.gpsimd.tensor_mul(q1[:], nmg[:], cnt1[:])
            ifu = sm.tile([P, TPH], F32, name=f"ifu_{h}")
            nc.gpsimd.tensor_mul(ifu[:], q1[:], ncnt2[:])
            q2 = sm.tile([P, TPH], F32, name=f"q2_{h}")
            nc.gpsimd.tensor_mul(q2[:], omf[:, acols], ifu[:])
            nc.gpsimd.tensor_add(iffiv[:, h * TPH : (h + 1) * TPH],
                                 q2[:], fin_t[:, acols])

            # tv[p1] <- 0, tv[i*] <- 0 where fire  (doesn't need r)
            u1 = work.tile([P, W], BF16, tag="u1", name=f"u1_{h}")
            nc.gpsimd.tensor_add(u1[:], ohp1[:], ohi[:])
            for t in range(TPH):
                ts = slice(t * S, (t + 1) * S)
                fcol = fire_a[:, h * TPH + t : h * TPH + t + 1]
                nw1 = gsc.tile([P, S], BF16, tag="nw1", name="nw1")
                nc.vector.tensor_scalar(nw1[:], u1[:][:, ts], fcol, 1.0,
                                        op0=OP.mult, op1=OP.not_equal)
                nc.vector.tensor_mul(tvt[:][:, ts], tvt[:][:, ts], nw1[:])
            nc.scalar.dma_start(
                tvo_d[rows, :].rearrange("(t p) s -> p t s", p=P),
                tvt[:].rearrange("p (t s) -> p t s", t=TPH))

            half_state.append((tdt, ohp0, rows))

        # ---- batched r computation ([P, NT], one Ln + one Exp table load)
        radd = sm.tile([P, NT], F32)
        nc.gpsimd.tensor_add(radd[:], h1_a[:], h0_a[:])
        rsub = sm.tile([P, NT], F32)
        nc.gpsimd.tensor_sub(rsub[:], h1_a[:], h0_a[:])
        rmul = sm.tile([P, NT], F32)
        nc.gpsimd.tensor_mul(rmul[:], h1_a[:], h0_a[:])
        den = sm.tile([P, NT], F32)
        nc.gpsimd.tensor_scalar_add(den[:], h0_a[:], 1e-7)
        rec = sm.tile([P, NT], F32)
        nc.vector.reciprocal(rec[:], den[:])
        rdiv = sm.tile([P, NT], F32)
        nc.gpsimd.tensor_mul(rdiv[:], h1_a[:], rec[:])
        base = sm.tile([P, NT], F32)
        nc.gpsimd.tensor_scalar_max(base[:], h1_a[:], 1e-7)
        lg = sm.tile([P, NT], F32)
        nc.scalar.activation(lg[:], base[:], ACTF.Ln)
        pm = sm.tile([P, NT], F32)
        nc.gpsimd.tensor_mul(pm[:], lg[:], h0_a[:])
        rpow = sm.tile([P, NT], F32)
        nc.scalar.activation(rpow[:], pm[:], ACTF.Exp)

        nc.vector.memset(r_a[:], 0.0)
        ohc = sm.tile([P, NT], U8, name="ohc", bufs=2)
        for vc, res in zip(vc_a, [radd, rsub, rmul, rdiv, rpow]):
            nc.vector.tensor_tensor(ohc[:], vc[:], vmax_a[:], op=OP.is_equal)
            nc.vector.copy_predicated(r_a[:], ohc[:], res[:])

        nc.scalar.dma_start(io_d.ap().rearrange("(p q) -> p q", p=P), iffiv[:])

        # ---- td scatter (needs r) + store
        for h, (tdt, ohp0, rows) in enumerate(half_state):
            for t in range(TPH):
                ts = slice(t * S, (t + 1) * S)
                cidx = h * TPH + t
                fcol = fire_a[:, cidx : cidx + 1]
                rcol = r_a[:, cidx : cidx + 1]
                w0 = gsc.tile([P, S], U8, tag="w0", name="w0")
                nc.vector.tensor_scalar(w0[:], ohp0[:][:, ts], fcol, None,
                                        op0=OP.mult)
                rb = gsc.tile([P, S], F32, tag="rb", name="rb")
                nc.vector.tensor_scalar(rb[:], zero_bc[:], 0.0, rcol,
                                        op0=OP.mult, op1=OP.add)
                nc.vector.copy_predicated(tdt[:][:, ts], w0[:], rb[:])
            nc.scalar.dma_start(
                tdo_d[rows, :].rearrange("(t p) s -> p t s", p=P),
                tdt[:].rearrange("p (t s) -> p t s", t=TPH))

    nc.compile()
    return nc


_NC_CACHE = {}


def _get_nc(sp_zero: bool = True):
    if sp_zero not in _NC_CACHE:
        _NC_CACHE[sp_zero] = _build_nc(sp_zero)
    return _NC_CACHE[sp_zero]


def _make_in_maps(trans_valid, trans_dense, trans_op, if_finished, if_valid,
                  start_pos):
    tv = np.ascontiguousarray(np.asarray(trans_valid, np.float32))
    td = np.ascontiguousarray(np.asarray(trans_dense, np.float32))
    # layout-only: channel planes [7, B, S]
    op = np.ascontiguousarray(
        np.asarray(trans_op, np.float32).transpose(2, 0, 1))
    fin = np.asarray(if_finished, np.float32)
    val = np.asarray(if_valid, np.float32)
    sp = int(start_pos)
    act2 = np.ascontiguousarray(
        np.tile((np.arange(S) >= sp).astype(np.float32), TPH))
    in_maps = []
    for c in range(NCORES):
        rows = slice(c * BS, (c + 1) * BS)
        # fv[p, t] = fin[t*128+p] ; fv[p, NT+t] = val[t*128+p]
        fvc = np.concatenate(
            [fin[rows].reshape(NT, P).T, val[rows].reshape(NT, P).T], axis=1)
        in_maps.append({
            "tv": tv[rows], "td": td[rows],
            "op": np.ascontiguousarray(op[:, rows, :]),
            "fv": np.ascontiguousarray(fvc), "act2": act2,
        })
    return in_maps


def _unpack_outs(outs, trans_op):
    tv_out = np.concatenate([outs[c]["tv_out"] for c in range(NCORES)], axis=0)
    td_out = np.concatenate([outs[c]["td_out"] for c in range(NCORES)], axis=0)
    iff = np.empty(B, np.float32)
    iv = np.empty(B, np.float32)
    for c in range(NCORES):
        arr = outs[c]["iffiv"].reshape(P, 2 * NT)
        rows = slice(c * BS, (c + 1) * BS)
        iff[rows] = arr[:, 0:NT].T.reshape(BS)
        iv[rows] = arr[:, NT : 2 * NT].T.reshape(BS)
    return tv_out, td_out, np.asarray(trans_op, np.float32), iff, iv


def kernel(trans_valid, trans_dense, trans_op, if_finished, if_valid, start_pos):
    nc = _get_nc(int(start_pos) == 0)
    in_maps = _make_in_maps(trans_valid, trans_dense, trans_op, if_finished,
                            if_valid, start_pos)
    res = run_bass_kernel_spmd(nc, in_maps, core_ids=list(range(NCORES)))
    return _unpack_outs(res.results, trans_op)
